# revision 10
# baseline (speedup 1.0000x reference)
"""Trainium2 Bass kernel for 4D transposed convolution (stride 2, kernel 3^4,
64->64 channels, input [4,64,12^4] -> output [4,64,25^4]).

Strategy: sub-pixel phase decomposition. Output position o = 2i + k per dim;
each of the 16 output parity phases is a sum of shifted 64x64 channel matmuls
over the 81 kernel taps. Sharding: 8 cores = batch(4) x o1-halves(2).
Per core, o1 rows are processed in (even,odd) pairs: PSUM partitions 0-63
hold the even row's Cout, 64-127 the odd row's; the contraction dim packs
Cin x {lo plane, hi plane} = 128. Matmuls run in float32r (full-rate fp32).
"""
import sys
sys.path.insert(0, '/opt/trn_rl_repo')
import numpy as np

B, CIN, COUT, L, KK, S = 4, 64, 64, 12, 3, 2
P = L + 2            # 14 padded extent per spatial dim
PLANE = P * P * P    # 2744
PLANE_AL = PLANE + 16
OUT = (L - 1) * S + KK   # 25
O3 = OUT * OUT * OUT     # 15625
NPL = 8              # planes per core slab
NO1 = 13             # o1 rows per core
N_CORES = 8

TRACE = False
LAST_EXEC_NS = None

_built = None


def _parities():
    out = []
    for p2 in (0, 1):
        for p3 in (0, 1):
            for p4 in (0, 1):
                out.append((p2, p3, p4))
    return out


def _taps(p2, p3, p4):
    k2s = [0, 2] if p2 == 0 else [1]
    k3s = [0, 2] if p3 == 0 else [1]
    k4s = [0, 2] if p4 == 0 else [1]
    for k2 in k2s:
        for k3 in k3s:
            for k4 in k4s:
                off = (196 if k2 in (0, 1) else 0) \
                    + (14 if k3 in (0, 1) else 0) \
                    + (1 if k4 in (0, 1) else 0)
                yield (k2 * 9 + k3 * 3 + k4, off)


def _build():
    import concourse.bass as bass
    from concourse import bacc
    import concourse.mybir as mybir
    from concourse.tile import TileContext

    F32R, F32 = mybir.dt.float32r, mybir.dt.float32
    ACT_ID = mybir.ActivationFunctionType.Identity

    nc = bacc.Bacc(None, target_bir_lowering=False, debug=True)
    xp = nc.dram_tensor("xp", [CIN, NPL * PLANE], F32R, kind="ExternalInput")
    wp = nc.dram_tensor("wp", [128, 27 * 128], F32R, kind="ExternalInput")
    wp1 = nc.dram_tensor("wp1", [128, 27 * 64], F32R, kind="ExternalInput")
    bias = nc.dram_tensor("bias", [128], F32, kind="ExternalInput")
    out = nc.dram_tensor("out", [COUT, NO1 * O3], F32, kind="ExternalOutput")

    def mk(ap, off, dims, nparts=None):
        p0 = list(ap.ap[0])
        if nparts is not None:
            p0 = [p0[0], nparts]
        return bass.AP(tensor=ap.tensor, offset=ap.offset + off,
                       ap=[p0] + [list(d) for d in dims])

    def raw(ap, off, dims):
        # DRAM-side AP with fully custom dim order (no partition prefix)
        return bass.AP(tensor=ap.tensor, offset=ap.offset + off,
                       ap=[list(d) for d in dims])

    with TileContext(nc) as tc:
        with tc.tile_pool(name="wpool", bufs=1) as wpool, \
             tc.tile_pool(name="xpool", bufs=3) as xpool, \
             tc.tile_pool(name="spool", bufs=3) as spool, \
             tc.tile_pool(name="pp", bufs=6, space="PSUM") as pp, \
             tc.tile_pool(name="ppd", bufs=1, space="PSUM") as ppd:

            wt = wpool.tile([128, 27 * 128], F32R, tag="wt")
            nc.sync.dma_start(out=wt[:], in_=wp[:])
            wt1 = wpool.tile([128, 27 * 64], F32R, tag="wt1")
            nc.sync.dma_start(out=wt1[:], in_=wp1[:])
            bt = wpool.tile([128, 1], F32, tag="bt")
            nc.sync.dma_start(out=bt[:], in_=raw(bias[:], 0, [[1, 128], [1, 1]]))

            # dummy matmuls absorb the weight-DMA waits once, up front
            psd = ppd.tile([128, 2], F32, tag="psd")
            nc.tensor.matmul(psd[:], wt[:, 0:128], wt[:, 0:2],
                             start=True, stop=True)
            nc.tensor.matmul(psd[0:64, :], wt1[:, 0:64], wt1[:, 0:2],
                             start=True, stop=True)

            for t in range(7):          # 6 full pairs + leftover (t=6)
                leftover = (t == 6)
                xt = xpool.tile([128, PLANE_AL], F32R, tag="xt")
                # partitions 0-63 <- plane t (lo), 64-127 <- plane t+1 (hi)
                nc.sync.dma_start(
                    out=xt[:, 0:PLANE],
                    in_=raw(xp[:], t * PLANE,
                            [[PLANE, 2], [NPL * PLANE, CIN], [1, PLANE]]))
                nc.vector.memset(xt[:, PLANE:PLANE_AL].bitcast(F32), 0.0)

                mparts = 64 if leftover else 128
                wsrc, wstride = (wt1, 64) if leftover else (wt, 128)

                for p2 in (0, 1):
                    n2c = 13 if p2 == 0 else 12
                    st = spool.tile([128, 13 * 625], F32, tag="st")
                    for r0 in range(0, n2c, 2):
                        rows = min(2, n2c - r0)
                        for p3 in (0, 1):
                            n3c = 13 if p3 == 0 else 12
                            for p4 in (0, 1):
                                n4c = 13 if p4 == 0 else 12
                                n4r = 14 if p4 == 0 else 12
                                N = rows * n3c * n4r
                                ps = pp.tile([128, 364], F32, tag="ps")
                                taps = list(_taps(p2, p3, p4))
                                for i, (kap, off) in enumerate(taps):
                                    mv = mk(xt[:], r0 * 196 + off,
                                            [[196, rows], [14, n3c], [1, n4r]])
                                    nc.tensor.matmul(
                                        mk(ps[:], 0, [[1, N]], nparts=mparts),
                                        wsrc[:, kap * wstride:
                                             kap * wstride + mparts],
                                        mv,
                                        start=(i == 0), stop=(i == len(taps) - 1))
                                # evacuate PSUM -> staging with bias add,
                                # interleaving o3/o4 parities
                                nc.scalar.activation(
                                    mk(st[:], r0 * 625 + p3 * 25 + p4,
                                       [[625, rows], [50, n3c], [2, n4c]],
                                       nparts=mparts),
                                    mk(ps[:], 0,
                                       [[n3c * n4r, rows], [n4r, n3c], [1, n4c]],
                                       nparts=mparts),
                                    ACT_ID, bias=bt[0:mparts], scale=1.0)
                    # one DMA per (pair, p2, o1-half): contiguous 625-elem runs
                    nhalf = 1 if leftover else 2
                    for h in range(nhalf):
                        o1 = 12 if leftover else 2 * t + h
                        nc.sync.dma_start(
                            out=raw(out[:], o1 * O3 + p2 * 625,
                                    [[NO1 * O3, COUT], [1250, n2c], [1, 625]]),
                            in_=st[h * 64:(h + 1) * 64, 0:n2c * 625])
    nc.compile()
    return nc


def _host_pack(x, weight, bias_v):
    # x: [B, 64, 12,12,12,12] -> padded [B, 64, 14, 2744]
    xpad = np.zeros((B, CIN, P, P, P, P), dtype=np.float32)
    xpad[:, :, 1:13, 1:13, 1:13, 1:13] = x
    xpad = xpad.reshape(B, CIN, P, PLANE)

    # weights: wp [128, 27*128]  (rows: slot0=ci(lo plane), slot1=64+ci(hi))
    W = weight.astype(np.float32)
    wph = np.zeros((128, 27, 128), dtype=np.float32)
    for k2 in range(3):
        for k3 in range(3):
            for k4 in range(3):
                kap = k2 * 9 + k3 * 3 + k4
                wph[0:64, kap, 0:64] = W[:, :, 2, k2, k3, k4]      # even row, lo
                wph[64:128, kap, 0:64] = W[:, :, 0, k2, k3, k4]    # even row, hi
                wph[64:128, kap, 64:128] = W[:, :, 1, k2, k3, k4]  # odd row, hi
    wp1h = np.zeros((128, 27, 64), dtype=np.float32)
    wp1h[0:64] = wph[0:64, :, 0:64]
    wp1h[64:128] = wph[64:128, :, 0:64]

    bh = np.concatenate([bias_v, bias_v]).astype(np.float32)

    in_maps = []
    for c in range(N_CORES):
        b, half = c // 2, c % 2
        start = 0 if half == 0 else 6
        slab = np.ascontiguousarray(
            xpad[b, :, start:start + NPL].reshape(CIN, NPL * PLANE))
        in_maps.append({
            "xp": slab,
            "wp": np.ascontiguousarray(wph.reshape(128, 27 * 128)),
            "wp1": np.ascontiguousarray(wp1h.reshape(128, 27 * 64)),
            "bias": bh,
        })
    return in_maps


def kernel(x, weight, bias):
    global _built, LAST_EXEC_NS
    from concourse.bass_utils import run_bass_kernel_spmd

    x = np.asarray(x, dtype=np.float32)
    weight = np.asarray(weight, dtype=np.float32)
    bias_v = np.asarray(bias, dtype=np.float32)

    if _built is None:
        _built = _build()
    in_maps = _host_pack(x, weight, bias_v)
    res = run_bass_kernel_spmd(_built, in_maps, core_ids=list(range(N_CORES)),
                               trace=TRACE)
    LAST_EXEC_NS = res.exec_time_ns

    full = np.empty((B, COUT, OUT, OUT, OUT, OUT), dtype=np.float32)
    for c in range(N_CORES):
        b, half = c // 2, c % 2
        co = res.results[c]["out"].reshape(COUT, NO1, OUT, OUT, OUT)
        if half == 0:
            full[b, :, 0:13] = co
        else:
            full[b, :, 13:25] = co[:, 1:13]
    return full


# revision 11
# speedup vs baseline: 1.3833x; 1.3833x over previous
"""Trainium2 Bass kernel for 4D transposed convolution (stride 2, kernel 3^4,
64->64 channels, input [4,64,12^4] -> output [4,64,25^4]).

Strategy: sub-pixel phase decomposition. Output position o = 2i + k per dim;
each of the 16 output parity phases is a sum of shifted 64x64 channel matmuls
over the 81 kernel taps. Sharding: 8 cores = batch(4) x o1-halves(2).
Per core, o1 rows are processed in (even,odd) pairs: PSUM partitions 0-63
hold the even row's Cout, 64-127 the odd row's; the contraction dim packs
Cin x {lo plane, hi plane} = 128. Matmuls run in fp16 (1 cycle/row, fp32
accumulation in PSUM); set DTYPE='f32r' for a full-precision fallback.
"""
import sys
sys.path.insert(0, '/opt/trn_rl_repo')
import numpy as np

B, CIN, COUT, L, KK, S = 4, 64, 64, 12, 3, 2
P = L + 2            # 14 padded extent per spatial dim
PLANE = P * P * P    # 2744
PLANE_AL = PLANE + 16
OUT = (L - 1) * S + KK   # 25
O3 = OUT * OUT * OUT     # 15625
NPL = 8              # planes per core slab
NO1 = 13             # o1 rows per core
N_CORES = 8
SLAB = 8125          # 13 * 625, per-(o1,co,p2) output slab

DTYPE = 'f16'        # 'f16' | 'f32r'
TRACE = False
LAST_EXEC_NS = None

_built = None
_built_dtype = None


def _taps(p2, p3, p4):
    k2s = [0, 2] if p2 == 0 else [1]
    k3s = [0, 2] if p3 == 0 else [1]
    k4s = [0, 2] if p4 == 0 else [1]
    for k2 in k2s:
        for k3 in k3s:
            for k4 in k4s:
                off = (196 if k2 in (0, 1) else 0) \
                    + (14 if k3 in (0, 1) else 0) \
                    + (1 if k4 in (0, 1) else 0)
                yield (k2 * 9 + k3 * 3 + k4, off)


def _build():
    import concourse.bass as bass
    from concourse import bacc
    import concourse.mybir as mybir
    from concourse.tile import TileContext

    F32 = mybir.dt.float32
    DT = mybir.dt.float16 if DTYPE == 'f16' else mybir.dt.float32r
    ACT_ID = mybir.ActivationFunctionType.Identity

    nc = bacc.Bacc(None, target_bir_lowering=False, debug=True)
    xp = nc.dram_tensor("xp", [CIN, NPL * PLANE], DT, kind="ExternalInput")
    wp = nc.dram_tensor("wp", [128, 27 * 128], DT, kind="ExternalInput")
    wp1 = nc.dram_tensor("wp1", [128, 27 * 64], DT, kind="ExternalInput")
    bias = nc.dram_tensor("bias", [128], F32, kind="ExternalInput")
    # layout: [o1, co, p2, slab(13*625)]
    out = nc.dram_tensor("out", [NO1, COUT * 2 * SLAB], F32,
                         kind="ExternalOutput")

    def mk(ap, off, dims, nparts=None):
        p0 = list(ap.ap[0])
        if nparts is not None:
            p0 = [p0[0], nparts]
        return bass.AP(tensor=ap.tensor, offset=ap.offset + off,
                       ap=[p0] + [list(d) for d in dims])

    def raw(ap, off, dims):
        return bass.AP(tensor=ap.tensor, offset=ap.offset + off,
                       ap=[list(d) for d in dims])

    with TileContext(nc) as tc:
        with tc.tile_pool(name="wpool", bufs=1) as wpool, \
             tc.tile_pool(name="xpool", bufs=3) as xpool, \
             tc.tile_pool(name="spool", bufs=3) as spool, \
             tc.tile_pool(name="pp", bufs=6, space="PSUM") as pp, \
             tc.tile_pool(name="ppd", bufs=1, space="PSUM") as ppd:

            wt = wpool.tile([128, 27 * 128], DT, tag="wt")
            nc.sync.dma_start(out=wt[:], in_=wp[:])
            wt1 = wpool.tile([128, 27 * 64], DT, tag="wt1")
            nc.sync.dma_start(out=wt1[:], in_=wp1[:])
            bt = wpool.tile([128, 1], F32, tag="bt")
            nc.sync.dma_start(out=bt[:], in_=raw(bias[:], 0, [[1, 128], [1, 1]]))

            # dummy matmuls absorb the weight-DMA waits once, up front
            psd = ppd.tile([128, 2], F32, tag="psd")
            nc.tensor.matmul(psd[:], wt[:, 0:128], wt[:, 0:2],
                             start=True, stop=True)
            nc.tensor.matmul(psd[0:64, :], wt1[:, 0:64], wt1[:, 0:2],
                             start=True, stop=True)

            for t in range(7):          # 6 full pairs + leftover (t=6)
                leftover = (t == 6)
                xt = xpool.tile([128, PLANE_AL], DT, tag="xt")
                # partitions 0-63 <- plane t (lo), 64-127 <- plane t+1 (hi)
                nc.sync.dma_start(
                    out=xt[:, 0:PLANE],
                    in_=raw(xp[:], t * PLANE,
                            [[PLANE, 2], [NPL * PLANE, CIN], [1, PLANE]]))
                nc.vector.memset(xt[:, PLANE:PLANE_AL].bitcast(F32), 0.0)

                mparts = 64 if leftover else 128
                wsrc, wstride = (wt1, 64) if leftover else (wt, 128)

                for p2 in (0, 1):
                    n2c = 13 if p2 == 0 else 12
                    st = spool.tile([128, SLAB], F32, tag="st")
                    for r0 in range(0, n2c, 2):
                        rows = min(2, n2c - r0)
                        for p3 in (0, 1):
                            n3c = 13 if p3 == 0 else 12
                            for p4 in (0, 1):
                                n4c = 13 if p4 == 0 else 12
                                n4r = 14 if p4 == 0 else 12
                                N = rows * n3c * n4r
                                ps = pp.tile([128, 364], F32, tag="ps")
                                taps = list(_taps(p2, p3, p4))
                                for i, (kap, off) in enumerate(taps):
                                    mv = mk(xt[:], r0 * 196 + off,
                                            [[196, rows], [14, n3c], [1, n4r]])
                                    nc.tensor.matmul(
                                        mk(ps[:], 0, [[1, N]], nparts=mparts),
                                        wsrc[:, kap * wstride:
                                             kap * wstride + mparts],
                                        mv,
                                        start=(i == 0), stop=(i == len(taps) - 1))
                                # evacuate PSUM -> staging with bias add,
                                # interleaving o3/o4 parities; split ACT/DVE
                                dst = mk(st[:], r0 * 625 + p3 * 25 + p4,
                                         [[625, rows], [50, n3c], [2, n4c]],
                                         nparts=mparts)
                                src = mk(ps[:], 0,
                                         [[n3c * n4r, rows], [n4r, n3c],
                                          [1, n4c]], nparts=mparts)
                                if (p3 + p4) % 2 == 0:
                                    nc.scalar.activation(dst, src, ACT_ID,
                                                         bias=bt[0:mparts],
                                                         scale=1.0)
                                else:
                                    nc.vector.tensor_scalar_add(
                                        dst, src, bt[0:mparts])
                    # one DMA per (pair, p2): full 128 partitions, 3-dim AP
                    if leftover:
                        nc.sync.dma_start(
                            out=raw(out[:], (12 * COUT * 2 + p2) * SLAB,
                                    [[2 * SLAB, COUT], [1, n2c * 625]]),
                            in_=st[0:64, 0:n2c * 625])
                    else:
                        q = nc.sync if p2 == 0 else nc.scalar
                        q.dma_start(
                            out=raw(out[:], (2 * t * COUT * 2 + p2) * SLAB,
                                    [[COUT * 2 * SLAB, 2], [2 * SLAB, COUT],
                                     [1, n2c * 625]]),
                            in_=st[:, 0:n2c * 625])
    nc.compile()
    return nc


def _host_pack(x, weight, bias_v):
    npdt = np.float16 if DTYPE == 'f16' else np.float32
    # x: [B, 64, 12,12,12,12] -> padded [B, 64, 14, 2744]
    xpad = np.zeros((B, CIN, P, P, P, P), dtype=npdt)
    xpad[:, :, 1:13, 1:13, 1:13, 1:13] = x
    xpad = xpad.reshape(B, CIN, P, PLANE)

    # weights: wp [128, 27*128]  (rows: slot0=ci(lo plane), slot1=64+ci(hi))
    W = weight.astype(np.float32)
    wph = np.zeros((128, 27, 128), dtype=npdt)
    k = W.reshape(CIN, COUT, 3, 27)
    wph[0:64, :, 0:64] = k[:, :, 2].transpose(0, 2, 1)      # even row, lo
    wph[64:128, :, 0:64] = k[:, :, 0].transpose(0, 2, 1)    # even row, hi
    wph[64:128, :, 64:128] = k[:, :, 1].transpose(0, 2, 1)  # odd row, hi
    wp1h = np.zeros((128, 27, 64), dtype=npdt)
    wp1h[0:64] = wph[0:64, :, 0:64]
    wp1h[64:128] = wph[64:128, :, 0:64]

    bh = np.concatenate([bias_v, bias_v]).astype(np.float32)

    wph = np.ascontiguousarray(wph.reshape(128, 27 * 128))
    wp1h = np.ascontiguousarray(wp1h.reshape(128, 27 * 64))
    in_maps = []
    for c in range(N_CORES):
        b, half = c // 2, c % 2
        start = 0 if half == 0 else 6
        slab = np.ascontiguousarray(
            xpad[b, :, start:start + NPL].reshape(CIN, NPL * PLANE))
        in_maps.append({"xp": slab, "wp": wph, "wp1": wp1h, "bias": bh})
    return in_maps


def kernel(x, weight, bias):
    global _built, _built_dtype, LAST_EXEC_NS
    from concourse.bass_utils import run_bass_kernel_spmd

    x = np.asarray(x, dtype=np.float32)
    weight = np.asarray(weight, dtype=np.float32)
    bias_v = np.asarray(bias, dtype=np.float32)

    if _built is None or _built_dtype != DTYPE:
        _built = _build()
        _built_dtype = DTYPE
    in_maps = _host_pack(x, weight, bias_v)
    res = run_bass_kernel_spmd(_built, in_maps, core_ids=list(range(N_CORES)),
                               trace=TRACE)
    LAST_EXEC_NS = res.exec_time_ns

    full = np.empty((B, COUT, OUT, OUT, OUT, OUT), dtype=np.float32)
    for c in range(N_CORES):
        b, half = c // 2, c % 2
        o1_lo = 0 if half == 0 else 12
        lo = 0 if half == 0 else 1
        co = res.results[c]["out"].reshape(NO1, COUT, 2, SLAB)
        for p2 in (0, 1):
            n2c = 13 if p2 == 0 else 12
            blk = co[lo:, :, p2, 0:n2c * 625].reshape(
                NO1 - lo, COUT, n2c, OUT, OUT)
            full[b, :, o1_lo + lo:o1_lo + NO1, p2::2] = \
                blk.transpose(1, 0, 2, 3, 4)
    return full
